# revision 16
# baseline (speedup 1.0000x reference)
"""Trainium2 Bass kernel for nn_ConvPlus1d (dense_cnn).

Algorithm (mathematically identical to the reference, derived analytically):

  The reference synthesizes per-sample conv weights:
      kern[b]   = mean_L(depthwise_conv(x))        -> [B, C_IN, K]
      w_in[b]   = W_in @ kern[b]                   -> [B, C_IN, K]
      w_out[b]  = <W_out, kern[b]>                 -> [B, C_OUT]
      bias[b]   = <W_bias, kern[b]>                -> [B, C_OUT]
      weight[b, o, c, k] = w_in[b, c, k] * w_out[b, o]     (rank-1!)
      y[b] = conv1d(x[b], weight[b], pad=1) + bias[b]

  Exact simplifications used here:
  1) mean over L of a pad-1 depthwise conv only needs per-channel sums and
     the first/last elements:  sum_l xpad[c, l+t] = {S-E, S, S-F}[t]
     where S = sum_l x[c,l], F = x[c,0], E = x[c,L-1].
     So (w_in | w_out | bias) is LINEAR in the stats (S, E, F) with
     coefficient matrices precomputable on the host:
         params[1, 448] = [S;E]^T @ M2 + [F]^T @ M1
     (param layout: w_in k-major [0:192] | w_out [192:320] | bias [320:448])
  2) The per-sample conv weight is rank-1 across (o) x (c,k).

  Device program per sample (data-parallel over batch, 4 samples/core):
      xs[0:64]   <- DMA x (bf16, host-padded to L+2)
      xs[64:128] <- DVE shifted copy of xs[0:64] (tap-1 rows)
      stats      <- DVE reduce_sum + first/last col picks
      params     <- 2 bf16 matmuls ([128,1]/[64,1] stationary -> [1,448])
      bias col   <- PE transpose of params[320:448]
      wtap01/wtap2 <- 3 rank-1 outer-product matmuls, evicted to bf16
      conv: 16 tiles of 512; per tile 2 matmuls (taps 0+1 stacked into a
            128-row contraction; tap 2 as 64-row) accumulating in psum
      eviction with bias add split across DVE / ACT / GPSIMD, bf16 out
      y out via DMA chunks on 3 queues (sync / scalar / gpsimd)

Sharding: batch 32 -> 8 cores x 4 samples, maker params replicated.
Host converts x to padded bf16 and y back to fp32 (pure glue; all math on
device). bf16 I/O halves HBM traffic vs fp32.
"""

import sys

import numpy as np

sys.path.insert(0, "/opt/trn_rl_repo")

import ml_dtypes  # noqa: E402

import concourse.bacc as bacc  # noqa: E402
import concourse.tile as tile  # noqa: E402
from concourse import mybir  # noqa: E402
from concourse.bass_utils import run_bass_kernel_spmd  # noqa: E402

B, C_IN, C_OUT, K, L = 32, 64, 128, 3, 8192
N_CORES = 8
BS = B // N_CORES          # samples per core
NT = 512                   # matmul moving-dim tile (one PSUM bank of fp32)
NTILES = L // NT           # 16

F32 = mybir.dt.float32
F32R = mybir.dt.float32r
BF16 = mybir.dt.bfloat16
F16 = mybir.dt.float16

# eviction engine per conv tile: A=scalar(ACT), D=vector(DVE).
# (gpsimd/Pool cannot read PSUM on TRN2 - it is an SBUF-to-SBUF DSP - and its
# software tensor ops are ~3.5ns/col, so it only issues DMAs.)
EVICT_PLAN = "ADAADAADAADAADAA"
# All data DMAs go through the sync engine's hardware-DGE queue: hw-dge
# spreads descriptors round-robin over all 16 DMA engines, so one queue
# gives full bandwidth; the gpsimd queue is software-DGE (~10us latency).
YQ_PLAN = ["sync", "sync", "sync", "sync"]


def _host_precompute(W_kernel, W_in, W_out, W_bias):
    """Fold maker parameters into linear maps M2 [128,448], M1 [64,448]."""
    Wk3 = W_kernel.reshape(C_IN, K, K).astype(np.float64)    # [c', j, t]
    a_S = (Wk3[:, :, 0] + Wk3[:, :, 1] + Wk3[:, :, 2]) / L   # coeff on S
    a_E = -Wk3[:, :, 0] / L                                  # coeff on E
    a_F = -Wk3[:, :, 2] / L                                  # coeff on F

    Win = W_in[:, :, 0].astype(np.float64)                   # [c, c']
    Wo = W_out.astype(np.float64)                            # [o, c', j]
    Wb = W_bias.astype(np.float64)                           # [o, c', j]

    def block(a):  # a: [c', j] -> [c'(stat), 448] coefficients
        m = np.zeros((C_IN, 448))
        # w_in part, k-major: param[j*64 + c] = sum_c' Win[c, c'] kern[c', j]
        for j in range(K):
            m[:, j * C_IN:(j + 1) * C_IN] = a[:, j:j + 1] * Win.T
        # w_out part
        m[:, 192:320] = np.einsum("ocj,cj->co", Wo, a)
        # bias part
        m[:, 320:448] = np.einsum("ocj,cj->co", Wb, a)
        return m

    M2 = np.concatenate([block(a_S), block(a_E)], axis=0)    # [128, 448]
    M1 = block(a_F)                                          # [64, 448]
    return M2.astype(np.float32), M1.astype(np.float32)


_CACHE = {}


def _build_module():
    if "nc" in _CACHE:
        return _CACHE["nc"]
    nc = bacc.Bacc("TRN2", target_bir_lowering=False, debug=False)

    x_d = nc.dram_tensor("x", [BS, C_IN, L + 2], BF16,
                         kind="ExternalInput").ap()
    m2_d = nc.dram_tensor("m2", [128, 448], BF16, kind="ExternalInput").ap()
    m1_d = nc.dram_tensor("m1", [C_IN, 448], BF16, kind="ExternalInput").ap()
    y_d = nc.dram_tensor("y", [BS, C_OUT, L], BF16,
                         kind="ExternalOutput").ap()

    with tile.TileContext(nc) as tc:
        with (
            tc.tile_pool(name="consts", bufs=1) as consts,
            tc.tile_pool(name="xp", bufs=4) as xp,
            tc.tile_pool(name="yp", bufs=3) as yp,
            tc.tile_pool(name="small", bufs=2) as small,
            tc.tile_pool(name="ps_y", bufs=6, space="PSUM") as psy,
            tc.tile_pool(name="ps_a", bufs=1, space="PSUM") as psa,
            tc.tile_pool(name="ps_b", bufs=1, space="PSUM") as psb,
        ):
            m2 = consts.tile([128, 448], BF16)
            m1 = consts.tile([C_IN, 448], BF16)
            ident = consts.tile([1, 1], F32)
            # consts go on the scalar queue so the sync queue's first issue
            # is sample 0's x (startup latency); scalar is otherwise unused
            # for DMA
            nc.scalar.dma_start(m2[:], m2_d)
            nc.scalar.dma_start(m1[:], m1_d)
            nc.vector.memset(ident[:], 1.0)

            state = {}

            def emit_sample_head(b):
                """DMA x, shifted copy, stats, synthesis for sample b."""
                xs = xp.tile([128, L + 2], BF16, tag="xs")
                # x in two pieces; piece boundary 4097 matches the fold
                # split so each half folds as soon as it lands
                nc.sync.dma_start(xs[0:C_IN, 0:4097], x_d[b][:, 0:4097])
                nc.sync.dma_start(xs[0:C_IN, 4097:L + 2],
                                  x_d[b][:, 4097:L + 2])
                # tap-1 rows via SBUF->SBUF DMA: xs[64+c, j] = xpad[c, j+1]
                # (zero compute-engine cost; DMA engines have headroom)
                nc.sync.dma_start(xs[C_IN:128, 0:L + 1],
                                  xs[0:C_IN, 1:L + 2])
                # stats: S = sum_l x[c, l] via fp16 tree folds (DVE 2-byte
                # fast mode; fp16 mantissa keeps fold error ~5e-4); per-half
                # folds start as soon as each x piece lands. Final 1024 cols
                # split DVE reduce / ACT accumulate.
                ta = small.tile([C_IN, 2048], F16, tag="ta")
                tb = small.tile([C_IN, 2048], F16, tag="tb")
                t2 = small.tile([C_IN, 2048], F16, tag="t2")
                t3 = small.tile([C_IN, 1024], F16, tag="t3")
                nc.vector.tensor_tensor(out=ta[:], in0=xs[0:C_IN, 1:2049],
                                        in1=xs[0:C_IN, 2049:4097],
                                        op=mybir.AluOpType.add)
                nc.vector.tensor_tensor(out=tb[:], in0=xs[0:C_IN, 4097:6145],
                                        in1=xs[0:C_IN, 6145:8193],
                                        op=mybir.AluOpType.add)
                nc.vector.tensor_tensor(out=t2[:], in0=ta[:], in1=tb[:],
                                        op=mybir.AluOpType.add)
                nc.vector.tensor_tensor(out=t3[:], in0=t2[:, 0:1024],
                                        in1=t2[:, 1024:2048],
                                        op=mybir.AluOpType.add)
                ssumd = small.tile([C_IN, 1], F32, tag="ssumd")
                ssuma = small.tile([C_IN, 1], F32, tag="ssuma")
                trash = small.tile([C_IN, 512], BF16, tag="trash")
                nc.vector.reduce_sum(out=ssumd[:], in_=t3[:, 0:512],
                                     axis=mybir.AxisListType.X)
                nc.scalar.activation(trash[:], t3[:, 512:1024],
                                     mybir.ActivationFunctionType.Copy,
                                     accum_out=ssuma[:])
                stat2 = small.tile([128, 1], BF16, tag="stat2")
                stat1 = small.tile([C_IN, 1], BF16, tag="stat1")
                nc.vector.tensor_tensor(out=stat2[0:C_IN, :], in0=ssumd[:],
                                        in1=ssuma[:], op=mybir.AluOpType.add)
                nc.vector.tensor_copy(stat2[C_IN:128, :], xs[0:C_IN, L:L + 1])
                nc.vector.tensor_copy(stat1[:], xs[0:C_IN, 1:2])
                state[b] = {"xs": xs, "stat2": stat2, "stat1": stat1}

            def emit_synth(b):
                """PE synthesis: params row, bias col, rank-1 conv weights."""
                st = state[b]
                bankA = psa.tile([128, 512], F32, tag="bankA")
                psp = bankA[0:1, 0:448]
                pb = bankA[:, 448:449]
                nc.tensor.matmul(psp, st["stat2"][:], m2[:],
                                 start=True, stop=False)
                nc.tensor.matmul(psp, st["stat1"][:], m1[:],
                                 start=False, stop=True)
                params = small.tile([1, 448], F32, tag="params")
                nc.scalar.copy(params[:], psp)
                paramsb = small.tile([1, 320], BF16, tag="paramsb")
                nc.vector.tensor_copy(paramsb[:], bankA[0:1, 0:320])

                nc.tensor.transpose(pb, params[0:1, 320:448], ident[:])
                bcol = small.tile([C_OUT, 1], F32, tag="bcol")
                nc.scalar.copy(bcol[:], pb)

                bankB = psb.tile([128, 256], F32, tag="bankB")
                wrow = paramsb[0:1, 192:320]
                for k in range(2):
                    nc.tensor.matmul(
                        bankB[64 * k:64 * (k + 1), 0:128],
                        paramsb[0:1, 64 * k:64 * (k + 1)],
                        wrow, start=True, stop=True)
                nc.tensor.matmul(bankB[0:C_IN, 128:256],
                                 paramsb[0:1, 128:192],
                                 wrow, start=True, stop=True)
                wtap01 = small.tile([128, C_OUT], BF16, tag="wtap01")
                wtap2 = small.tile([C_IN, C_OUT], BF16, tag="wtap2")
                nc.vector.tensor_copy(wtap01[:], bankB[:, 0:128])
                nc.scalar.copy(wtap2[:], bankB[0:C_IN, 128:256])
                st["wtap01"] = wtap01
                st["wtap2"] = wtap2
                st["bcol"] = bcol
                yb = yp.tile([C_OUT, L], BF16, tag="yb")
                st["yb"] = yb

            def emit_conv(b, tiles):
                """Conv matmuls + evictions + y DMA for given tile range."""
                st = state[b]
                xs, yb = st["xs"], st["yb"]
                pys = []
                for t in tiles:
                    py = psy.tile([C_OUT, NT], F32, tag="py", name=f"py_{b}_{t}")
                    pys.append(py)
                    nc.tensor.matmul(py[:], st["wtap01"][:],
                                     xs[:, NT * t:NT * t + NT],
                                     start=True, stop=False)
                for py, t in zip(pys, tiles):
                    nc.tensor.matmul(py[:], st["wtap2"][:],
                                     xs[0:C_IN, NT * t + 2:NT * t + 2 + NT],
                                     start=False, stop=True)
                for py, t in zip(pys, tiles):
                    sl = yb[:, NT * t:NT * (t + 1)]
                    eng = EVICT_PLAN[t]
                    if eng == "D":
                        nc.vector.tensor_scalar(
                            out=sl, in0=py[:], scalar1=st["bcol"][:],
                            scalar2=None, op0=mybir.AluOpType.add)
                    elif eng == "A":
                        nc.scalar.activation(
                            sl, py[:], mybir.ActivationFunctionType.Identity,
                            bias=st["bcol"][:], scale=1.0)
                    else:
                        raise AssertionError(f"bad evict engine {eng}")
                    # after the 4th tile of each 2048-col span, ship it
                    if t % 4 == 3:
                        c = t // 4
                        q = getattr(nc, YQ_PLAN[c])
                        q.dma_start(y_d[b][:, 2048 * c:2048 * (c + 1)],
                                    yb[:, 2048 * c:2048 * (c + 1)])

            # software pipeline: head(b+1) early (x DMA + stats overlap
            # conv b); synth(b+1) sits before conv b's tail so the tail's
            # 8 matmuls hide the wtap-eviction latency. Conv chunks are
            # <= 6 tiles so the 6-buffer psum ring never waits on an
            # eviction that is queued behind the PE.
            emit_sample_head(0)
            emit_sample_head(1)
            emit_synth(0)
            for b in range(BS):
                emit_conv(b, range(0, 6))
                if b + 2 < BS:
                    emit_sample_head(b + 2)
                emit_conv(b, range(6, 12))
                if b + 1 < BS:
                    emit_synth(b + 1)
                emit_conv(b, range(12, 16))
                state.pop(b - 1, None)

    nc.compile()
    _CACHE["nc"] = nc
    return nc


def kernel(x, W_kernel, W_in, W_out, W_bias):
    x = np.asarray(x, dtype=np.float32)
    # one zero column each side: the device reads xpad[l], xpad[l+1], xpad[l+2]
    xpad = np.pad(x, [(0, 0), (0, 0), (1, 1)]).astype(ml_dtypes.bfloat16)
    M2, M1 = _host_precompute(
        np.asarray(W_kernel, np.float32), np.asarray(W_in, np.float32),
        np.asarray(W_out, np.float32), np.asarray(W_bias, np.float32))
    M2 = M2.astype(ml_dtypes.bfloat16)
    M1 = M1.astype(ml_dtypes.bfloat16)

    nc = _build_module()
    in_maps = [
        {"x": xpad[c * BS:(c + 1) * BS], "m2": M2, "m1": M1}
        for c in range(N_CORES)
    ]
    res = run_bass_kernel_spmd(nc, in_maps, core_ids=list(range(N_CORES)))
    global LAST_RESULT
    LAST_RESULT = res
    y = np.concatenate([np.asarray(r["y"], dtype=np.float32)
                        for r in res.results], axis=0)
    return y


LAST_RESULT = None
